# revision 25
# baseline (speedup 1.0000x reference)
"""Causal multi-head self-attention on 8 Trainium2 NeuronCores.

Problem: B=4, T=2048, C=1024, H=16 heads (d=64), fp32.
    q/k/v = x @ W{q,k,v}.T + b;  S = causal softmax(q k^T / sqrt(d));  y = (S v) @ Wo.T + bo

Sharding (8 cores): 2-D  (batch x head-group).
    core c -> batch b = c // 2, head-group g = c % 2 (8 heads / 512 features).
    Each core computes its batch's attention for its 8 heads plus the partial
    output projection against Wo[:, 512g:512g+512]; the host sums the two
    partials per batch and adds bo.

Device kernel (per core, identical SPMD program, Bass/Tile):
    phase 1: q^T,k^T (features on partitions) and v (tokens on partitions)
        from x^T tiles streamed from DRAM in 512-token chunks.
    phase 2 (query-chunk outer, head-pair inner): S^T tiles for an even/odd
        head pair = k_j^T.T @ q^T into a 2-bank PSUM pair (the two K=64
        matmuls land on PE row-groups 0/64), one fused exp over both heads
        (ACT, PSUM->SBUF), causal masking via a single gpsimd affine_select
        over the masked+boundary columns (exp skips fully-masked columns).
        PV matmuls O^T += [v|1].T @ E run software-pipelined one j-step
        behind the S matmuls so the PE never waits on ACT. Row 64 of the PV
        accumulator is the softmax denominator; normalize with
        partition_broadcast (via a partition-0 DMA hop) + reciprocal.
    phase 3 (lagged one chunk behind attention): partial
        out = O_norm^T.T @ Wo_slice^T, DMA to DRAM.

All host-side work is layout only (transpose/slice/replicate) + the final
pairwise partial-sum; every FLOP of the reference runs on device except the
8M-element partial-sum adds.
"""

import math
import os

import numpy as np

# persistent XLA/neuronx compile cache: makes repeat kernel() invocations
# from fresh processes skip the ~5 min helper-module compile when possible.
os.environ.setdefault("JAX_COMPILATION_CACHE_DIR", "/tmp/jax_comp_cache")

B, T, C, H = 4, 2048, 1024, 16
D = C // H  # 64
NCORES = 8
GROUPS = 2  # head-groups (tensor parallel dimension)
HG = H // GROUPS  # heads per core = 8
CG = C // GROUPS  # features per core = 512
SCALE = 1.0 / math.sqrt(D)
P = 128
TCH = 512  # query chunk / matmul free dim
NTCH = T // TCH  # 4
NHP = CG // P  # 4 head-pairs per core

# float32 is exact but ~4 cycles/row on the PE; float32r runs ~1 cycle/row
# (measured 227 ns per 512-col matmul) at ~1.5e-4 relative precision.
_MODULE_CACHE = {}


def _build_module(mm_fast):
    import concourse.bass as bass  # noqa: F401
    import concourse.mybir as mybir
    import concourse.tile as tile
    from concourse import bacc

    f32 = mybir.dt.float32
    # matmul-feeding tensors use float32r when mm_fast (same fp32 bits in
    # DRAM/host). The BIR verifier requires every producer of an fp32r
    # matmul operand to write the float32r dtype, so the dtype is set on
    # the tensors themselves.
    mdt = mybir.dt.float32r if mm_fast else f32
    Exp = mybir.ActivationFunctionType.Exp

    nc = bacc.Bacc(None, target_bir_lowering=False)

    xt = nc.dram_tensor("xt", [C, T], mdt, kind="ExternalInput")
    wqt = nc.dram_tensor("wqt", [C, CG], mdt, kind="ExternalInput")
    wkt = nc.dram_tensor("wkt", [C, CG], mdt, kind="ExternalInput")
    wvt = nc.dram_tensor("wvt", [C, CG], mdt, kind="ExternalInput")
    wot = nc.dram_tensor("wot", [CG, C], mdt, kind="ExternalInput")
    bq2 = nc.dram_tensor("bq2", [P, NHP], f32, kind="ExternalInput")
    bk2 = nc.dram_tensor("bk2", [P, NHP], f32, kind="ExternalInput")
    bvb = nc.dram_tensor("bvb", [P, CG], f32, kind="ExternalInput")
    out = nc.dram_tensor("out", [T, C], f32, kind="ExternalOutput")

    xt_r = xt.ap().rearrange("(cs p) t -> p cs t", p=P)  # [128, 8, 2048]
    wqt_r = wqt.ap().rearrange("(cs p) j -> p cs j", p=P)  # [128, 8, 512]
    wkt_r = wkt.ap().rearrange("(cs p) j -> p cs j", p=P)
    wvt_r = wvt.ap().rearrange("(cs p) j -> p cs j", p=P)
    wot_r = wot.ap().rearrange("(hp p) m -> p hp m", p=P)  # [128, 4, 1024]
    out_ap = out.ap()

    with tile.TileContext(nc) as tc:
        with (
            tc.tile_pool(name="persist", bufs=1) as persist,
            tc.tile_pool(name="smalls", bufs=1) as smalls,
            tc.tile_pool(name="psO", bufs=2, space="PSUM") as psO,
            tc.tile_pool(name="ps3", bufs=2, space="PSUM") as ps3p,
        ):
            # per-chunk tensors so phase boundaries overlap under Tile deps
            qT_t = []  # [feature-partition, head-pair, token] per chunk
            kT_t = []
            vx_t = []  # [token-partition, token-tile, head, d+1] per chunk
            for c in range(NTCH):
                qT_t.append(persist.tile([P, NHP, TCH], mdt, name=f"qT{c}"))
                kT_t.append(persist.tile([P, NHP, TCH], mdt, name=f"kT{c}"))
                vx_t.append(
                    persist.tile([P, TCH // P, HG, D + 1], mdt, name=f"vx{c}")
                )

            bqs = smalls.tile([P, NHP], f32)
            nc.sync.dma_start(bqs, bq2.ap())
            bks = smalls.tile([P, NHP], f32)
            nc.sync.dma_start(bks, bk2.ap())
            bvbs = smalls.tile([P, CG], f32)
            nc.sync.dma_start(bvbs, bvb.ap())
            # ones columns of v_ext (softmax-denominator trick); memset
            # can't write float32r, so use a DVE tensor_scalar: 0*x + 1.
            for c in range(NTCH):
                nc.vector.tensor_scalar(
                    vx_t[c][:, :, :, D],
                    bvbs[:, 0 : (TCH // P) * HG].rearrange(
                        "p (a b) -> p a b", b=HG
                    ),
                    0.0,
                    1.0,
                    mybir.AluOpType.mult,
                    mybir.AluOpType.add,
                )

            # ---------------- phase 1: projections ----------------
            with (
                tc.tile_pool(name="p1w", bufs=1) as p1w,
                tc.tile_pool(name="p1x", bufs=2) as p1x,
            ):
                # x chunk 0 and the first half of Wq lead the DMA queue so
                # the first q accumulation chains start earlier than if all
                # 8MB of upfront loads queued ahead of them.
                xtt0 = p1x.tile([P, 8, TCH], mdt, tag="xtt", name="xtt0")
                wqts = p1w.tile([P, 8, CG], mdt)
                for cs2 in range(0, 8, 2):
                    nc.sync.dma_start(
                        xtt0[:, cs2 : cs2 + 2, :], xt_r[:, cs2 : cs2 + 2, 0:TCH]
                    )
                    nc.sync.dma_start(
                        wqts[:, cs2 : cs2 + 2, :], wqt_r[:, cs2 : cs2 + 2, :]
                    )
                wkts = p1w.tile([P, 8, CG], mdt)
                nc.sync.dma_start(wkts, wkt_r)
                wvts = p1w.tile([P, 8, CG], mdt)
                nc.sync.dma_start(wvts, wvt_r)

                for tch in range(NTCH):
                    tsl = slice(TCH * tch, TCH * (tch + 1))
                    if tch == 0:
                        xtt = xtt0
                    else:
                        xtt = p1x.tile([P, 8, TCH], mdt, tag="xtt", name=f"xtt{tch}")
                        nc.sync.dma_start(xtt, xt_r[:, :, tsl])
                    for jt in range(NHP):
                        jsl = slice(P * jt, P * (jt + 1))
                        psq = ps3p.tile([P, TCH], f32, tag="pso3", name=f"psq{tch}_{jt}")
                        for cs in range(8):
                            nc.tensor.matmul(
                                psq,
                                wqts[:, cs, jsl],
                                xtt[:, cs, :],
                                start=(cs == 0),
                                stop=(cs == 7),
                            )
                        nc.vector.tensor_scalar_add(
                            qT_t[tch][:, jt, :], psq, bqs[:, jt : jt + 1]
                        )
                        psk = ps3p.tile([P, TCH], f32, tag="pso3", name=f"psk{tch}_{jt}")
                        for cs in range(8):
                            nc.tensor.matmul(
                                psk,
                                wkts[:, cs, jsl],
                                xtt[:, cs, :],
                                start=(cs == 0),
                                stop=(cs == 7),
                            )
                        nc.vector.tensor_scalar_add(
                            kT_t[tch][:, jt, :], psk, bks[:, jt : jt + 1]
                        )
                    for tt in range(TCH // P):
                        gt = 4 * tch + tt  # global 128-token tile index
                        psv = ps3p.tile([P, CG], f32, tag="pso3", name=f"psv{tch}_{tt}")
                        for cs in range(8):
                            nc.tensor.matmul(
                                psv,
                                xtt[:, cs, P * tt : P * (tt + 1)],
                                wvts[:, cs, :],
                                start=(cs == 0),
                                stop=(cs == 7),
                            )
                        nc.vector.tensor_add(
                            vx_t[gt // 4][:, gt % 4, :, 0:D],
                            psv.rearrange("p (h d) -> p h d", d=D),
                            bvbs.rearrange("p (h d) -> p h d", d=D),
                        )

            # ---------------- phases 2+3 ----------------
            with (
                tc.tile_pool(name="otp", bufs=3) as otp,
                tc.tile_pool(name="ep", bufs=6) as ep,
                tc.tile_pool(name="npool", bufs=2) as npool,
                tc.tile_pool(name="psS", bufs=2, space="PSUM") as psS,
            ):
                wots = otp.tile([P, NHP, C], mdt, tag="wots", bufs=1)
                nc.sync.dma_start(wots, wot_r)
                ot_t = {}

                def attention(ic, hp):
                    if hp == 0:
                        ot_t[ic] = otp.tile(
                            [P, NHP, TCH], mdt, tag="ot", name=f"ot{ic}"
                        )
                    njt = 4 * (ic + 1)
                    ps_oe = psO.tile([P, TCH], f32, tag="ps_o", name=f"poe{ic}_{hp}")
                    ps_oo = psO.tile([P, TCH], f32, tag="ps_o", name=f"poo{ic}_{hp}")
                    ps_os = (ps_oe, ps_oo)
                    pend = []  # (jt, E) awaiting their PV matmuls

                    def flush_pv2():
                        # two j-steps at once, grouped per head so consecutive
                        # matmuls hit the same PSUM bank (cheaper than
                        # alternating banks every matmul).
                        grp, pend[:2] = pend[:2], []
                        for h01 in range(2):
                            for jt, ee in grp:
                                cj, lj = jt // 4, jt % 4
                                # columns < 128r of a diagonal tile are fully
                                # masked: zero contribution, so the PV matmul
                                # skips them (earlier j-tiles wrote them).
                                lo = max(0, P * (jt - 4 * ic))
                                nc.tensor.matmul(
                                    ps_os[h01][0 : D + 1, lo:],
                                    vx_t[cj][:, lj, 2 * hp + h01, :],
                                    ee[:, h01, lo:],
                                    start=(jt == 0),
                                    stop=(jt == njt - 1),
                                )

                    for jt in range(njt):
                        cj, lj = jt // 4, jt % 4
                        r = jt - 4 * ic  # >= 0 only for diagonal tiles
                        # columns < 128r are fully masked; shrink the S matmul
                        # to the live tail (but keep free dim >= 256: float32r
                        # runs 4x slower below that).
                        lo2 = 0 if r <= 0 else min(P * r, TCH // 2)
                        psp = psS.tile(
                            [P, 2, TCH], f32, tag="psp", name=f"psp{ic}_{hp}_{jt}"
                        )
                        for h01 in range(2):
                            pb = 64 * h01
                            nc.tensor.matmul(
                                psp[:, h01, lo2:],
                                kT_t[cj][pb : pb + D, hp, P * lj : P * (lj + 1)],
                                qT_t[ic][pb : pb + D, hp, lo2:],
                                start=True,
                                stop=True,
                            )
                        ee = ep.tile(
                            [P, 2, TCH], mdt, tag="ee", name=f"ee{ic}_{hp}_{jt}"
                        )
                        if r <= 0:
                            nc.scalar.activation(ee, psp, Exp, scale=SCALE)
                        else:
                            # columns < 128r are fully masked: never computed
                            # (the PV matmul skips them too).
                            nc.scalar.activation(
                                ee[:, :, P * r :],
                                psp[:, :, P * r :],
                                Exp,
                                scale=SCALE,
                            )
                        if r >= 0:
                            # boundary 128 columns: keep where -p + f >= 0
                            # (f local to the slice starting at column 128r)
                            bsl = slice(P * r, P * (r + 1))
                            nc.gpsimd.affine_select(
                                out=ee[:, :, bsl],
                                in_=ee[:, :, bsl],
                                compare_op=mybir.AluOpType.is_ge,
                                fill=0.0,
                                base=0,
                                pattern=[[0, 2], [1, P]],
                                channel_multiplier=-1,
                            )
                        pend.append((jt, ee))
                        if len(pend) == 4:
                            flush_pv2()
                    while pend:
                        flush_pv2()

                    # Evacuate the PV accumulators to SBUF right away so the
                    # PSUM banks recycle without waiting on the (high-latency)
                    # normalization chain; the normalization itself is emitted
                    # one head-pair later so the PE stream never pauses.
                    oraw = []
                    for h01 in range(2):
                        ow = npool.tile(
                            [D + 1, TCH], f32, tag=f"oraw{h01}", name=f"or{ic}_{hp}_{h01}"
                        )
                        nc.vector.tensor_copy(ow, ps_os[h01][0 : D + 1, :])
                        oraw.append(ow)
                    return oraw

                def normalize(ic, hp, oraw):
                    # rows 0..63 are O^T, row 64 the softmax sums.
                    # partition_broadcast only reads physical partition 0
                    # (base-64 APs return garbage on HW): DMA-hop the row.
                    for h01 in range(2):
                        ow = oraw[h01]
                        stmp = npool.tile(
                            [1, TCH], f32, tag="stmp", name=f"st{ic}_{hp}_{h01}"
                        )
                        nc.sync.dma_start(stmp, ow[D : D + 1, :])
                        rb = npool.tile(
                            [D, TCH], f32, tag="rb", name=f"rb{ic}_{hp}_{h01}"
                        )
                        nc.gpsimd.partition_broadcast(rb, stmp)
                        if mm_fast:
                            nc.vector.reciprocal_approx_fast(rb, rb)
                        else:
                            nc.vector.reciprocal(rb, rb)
                        if h01 == 0:
                            nc.vector.tensor_mul(
                                ot_t[ic][0:D, hp, :], ow[0:D, :], rb
                            )
                        else:
                            tmpn = npool.tile(
                                [D, TCH], mdt, tag="tmpn", name=f"tn{ic}_{hp}"
                            )
                            nc.vector.tensor_mul(tmpn, ow[0:D, :], rb)
                            nc.sync.dma_start(ot_t[ic][D:P, hp, :], tmpn)

                def outproj(ic):
                    otn = ot_t.pop(ic)
                    for tt in range(TCH // P):
                        trow = TCH * ic + P * tt
                        for mi in range(C // TCH):
                            msl = slice(TCH * mi, TCH * (mi + 1))
                            pso3 = ps3p.tile(
                                [P, TCH], f32, tag="pso3", name=f"ps3{ic}_{tt}_{mi}"
                            )
                            for hp in range(NHP):
                                nc.tensor.matmul(
                                    pso3,
                                    otn[:, hp, P * tt : P * (tt + 1)],
                                    wots[:, hp, msl],
                                    start=(hp == 0),
                                    stop=(hp == NHP - 1),
                                )
                            osb = ep.tile(
                                [P, TCH], f32, tag="osb", name=f"ob{ic}_{tt}_{mi}"
                            )
                            nc.vector.tensor_copy(osb, pso3)
                            nc.sync.dma_start(out_ap[trow : trow + P, msl], osb)

                # normalization lags attention by one head-pair and the
                # out-projection by one chunk, so the PE stream never stalls
                # on the normalization chain's DMA/gpsimd latency.
                norm_q = []
                for ic in range(NTCH):
                    for hp in range(NHP):
                        norm_q.append((ic, hp, attention(ic, hp)))
                        if len(norm_q) >= 2:
                            normalize(*norm_q.pop(0))
                    if ic >= 1:
                        outproj(ic - 1)
                while norm_q:
                    normalize(*norm_q.pop(0))
                outproj(NTCH - 1)

    nc.compile()
    return nc


def get_module(mm_fast=True):
    key = bool(mm_fast)
    if key not in _MODULE_CACHE:
        _MODULE_CACHE[key] = _build_module(key)
    return _MODULE_CACHE[key]


def make_in_maps(x, Wq, bq, Wk, bk, Wv, bv, Wo, bo):
    x = np.asarray(x, dtype=np.float32)
    Wq = np.asarray(Wq, dtype=np.float32)
    Wk = np.asarray(Wk, dtype=np.float32)
    Wv = np.asarray(Wv, dtype=np.float32)
    Wo = np.asarray(Wo, dtype=np.float32)
    bq = np.asarray(bq, dtype=np.float32)
    bk = np.asarray(bk, dtype=np.float32)
    bv = np.asarray(bv, dtype=np.float32)

    in_maps = []
    for core in range(NCORES):
        b, g = core // GROUPS, core % GROUPS
        gs = slice(CG * g, CG * (g + 1))
        in_maps.append(
            {
                "xt": np.ascontiguousarray(x[b].T),
                "wqt": np.ascontiguousarray(Wq[gs, :].T),
                "wkt": np.ascontiguousarray(Wk[gs, :].T),
                "wvt": np.ascontiguousarray(Wv[gs, :].T),
                "wot": np.ascontiguousarray(Wo[:, gs].T),
                "bq2": np.ascontiguousarray(bq[gs].reshape(NHP, P).T),
                "bk2": np.ascontiguousarray(bk[gs].reshape(NHP, P).T),
                "bvb": np.ascontiguousarray(
                    np.broadcast_to(bv[gs][None, :], (P, CG))
                ),
            }
        )
    return in_maps


def combine_results(results, bo):
    bo = np.asarray(bo, dtype=np.float32)
    out = np.empty((B, T, C), dtype=np.float32)
    for b in range(B):
        out[b] = (
            results[GROUPS * b]["out"]
            + results[GROUPS * b + 1]["out"]
            + bo[None, :]
        )
    return out


def kernel(**inputs):
    from concourse.bass_utils import run_bass_kernel_spmd

    nc = get_module(mm_fast=True)
    in_maps = make_in_maps(
        inputs["x"],
        inputs["Wq"],
        inputs["bq"],
        inputs["Wk"],
        inputs["bk"],
        inputs["Wv"],
        inputs["bv"],
        inputs["Wo"],
        inputs["bo"],
    )
    res = run_bass_kernel_spmd(nc, in_maps, core_ids=list(range(NCORES)))
    return combine_results(res.results, inputs["bo"])
